# revision 5
# baseline (speedup 1.0000x reference)
"""GQA attention (RoPE, causal, per-head q-scale) on 8 TRN2 NeuronCores.

Sharding: 2-way data-parallel over batch x 4-way tensor-parallel over heads.
Core c handles batch b=c//4 and head group g=c%4 (8 q heads, 2 kv heads).
Each core computes qkv-proj -> rope -> causal attention -> partial o_proj
(over its heads' columns of Wo); the host sums the 4 partials per batch.

All scalar factors (rope_mscale, sm_scale, per_head_scale) are folded into
the Wq/Wk rows on the host. Causal masking is applied on-chip by PSUM
matmul accumulation of -BIG (rank-1 for fully-masked blocks, identity x
lower-triangular constant for diagonal blocks) before the exp.

Layouts on device (partition, free):
  xt      [hid, s]        hidden^T, streamed in 256-col chunks
  wqkv    [hid, 768]      [Wq(8 heads, scaled) | Wk(2 kv, scaled) | Wv].T
  q/k^T   [d*heads, s]    head-major rows; rope applied in this layout
  scores^T[sk, sq]        per (head, sk-chunk 128, sq-chunk 512) in PSUM
  exp^T   [sk, sq]        SBUF, fed as matmul rhs
  Vaug    [sk, 65]        V rows + ones column (row 64 of out accumulates
                          the softmax denominators)
  out^T   [d+1, sq]       PSUM accumulator per (head, sq-chunk)
  attn^T  [o(=2 heads), s] normalized, lhsT for o_proj
  out     [s, hid_out]    partial o_proj result, one per core
"""

import sys, os

for _p in ("/opt/trn_rl_repo", "/root/.axon_site/_ro/trn_rl_repo"):
    if os.path.isdir(_p) and _p not in sys.path:
        sys.path.insert(0, _p)

import numpy as np

import concourse.bass as bass
import concourse.mybir as mybir
import concourse.tile as tile
from concourse import bacc
from concourse.bass_utils import run_bass_kernel_spmd

F32 = mybir.dt.float32
AF = mybir.ActivationFunctionType

B, S, HID = 2, 2048, 2048
H, K, D = 32, 8, 64
G = H // K
ROPE_MSCALE = 1.2
SM_SCALE = 1.0 / (D ** 0.5)
BIG = 30000.0

NH = 8          # q heads per core
NKV = 2         # kv heads per core
NPAIR = 4       # q head pairs per core
QO = NH * D     # 512 q rows
NK = HID // 128  # 16 contraction chunks
XW = 256        # xt streaming chunk width
NN = S // XW    # 8 xt chunks
SQW = 512       # attention sq chunk width
NJ = S // SQW   # 4 sq chunks
NSK = S // 128  # 16 sk chunks

_CACHED = {}


def _build():
    if "nc" in _CACHED:
        return _CACHED["nc"]

    nc = bacc.Bacc(None)

    xt_d = nc.declare_dram_parameter("xt", [HID, S], F32, isOutput=False)
    wqkv_d = nc.declare_dram_parameter("wqkv", [HID, 768], F32, isOutput=False)
    wo_d = nc.declare_dram_parameter("wo", [QO, HID], F32, isOutput=False)
    cost_d = nc.declare_dram_parameter("cost", [64, S], F32, isOutput=False)
    sints_d = nc.declare_dram_parameter("sints", [64, S], F32, isOutput=False)
    consts_d = nc.declare_dram_parameter("consts", [128, 896], F32, isOutput=False)
    out_d = nc.declare_dram_parameter("out", [S, HID], F32, isOutput=True)

    with tile.TileContext(nc) as tc:
        # ---------- long-lived pools ----------
        with (
            tc.tile_pool(name="consts", bufs=1) as consts_pool,
            tc.tile_pool(name="ktv", bufs=1) as ktv_pool,
            tc.tile_pool(name="qrope", bufs=8) as qrope_pool,
            tc.tile_pool(name="expt", bufs=5) as expt_pool,
            tc.tile_pool(name="attnt", bufs=16) as attnt_pool,
            tc.tile_pool(name="inv", bufs=2) as inv_pool,
            tc.tile_pool(name="psc", bufs=3, space="PSUM") as psc_pool,
            tc.tile_pool(name="pout2", bufs=2, space="PSUM") as pout2_pool,
            tc.tile_pool(name="pbc", bufs=1, space="PSUM") as pbc_pool,
        ):
            consts = consts_pool.tile([128, 896], F32, name="consts")
            nc.sync.dma_start(out=consts, in_=consts_d[:, :])
            ident = consts[:, 0:128]
            tri = consts[:, 128:256]
            ones_row = consts[0:1, 256:384]   # [1,128] of 1.0
            big_row = consts[0:1, 384:896]    # [1,512] of -BIG

            kt_aa = ktv_pool.tile([128, S], F32, tag="ktaa", name="ktaa")
            kt_bb = ktv_pool.tile([128, S], F32, tag="ktbb", name="ktbb")
            vaug = [
                ktv_pool.tile([128, NSK, 65], F32, tag=f"vaug{i}", name=f"vaug{i}") for i in range(NKV)
            ]
            for i in range(NKV):
                nc.vector.memset(vaug[i][:, :, 64:65], 1.0)

            qrope = {}   # (m, j) -> tile [128, SQW]
            attnt = {}   # (m, j) -> tile [128, SQW]

            # ================= era 1: qkv proj + rope + attention =================
            with (
                tc.tile_pool(name="wq", bufs=1) as wq_pool,
                tc.tile_pool(name="xt", bufs=3) as xt_pool,
                tc.tile_pool(name="cs", bufs=1) as cs_pool,
                tc.tile_pool(name="vt", bufs=2) as vt_pool,
                tc.tile_pool(name="rtmp", bufs=2) as rtmp_pool,
                tc.tile_pool(name="pqkv", bufs=2, space="PSUM") as pqkv_pool,
            ):
                wqt = wq_pool.tile([128, NK, 768], F32, name="wqt")
                for k in range(NK):
                    nc.sync.dma_start(
                        out=wqt[:, k, :], in_=wqkv_d[k * 128:(k + 1) * 128, :]
                    )
                cost = cs_pool.tile([64, S], F32, tag="cost", name="cost")
                sints = cs_pool.tile([64, S], F32, tag="sints", name="sints")
                nc.sync.dma_start(out=cost, in_=cost_d[:, :])
                nc.sync.dma_start(out=sints, in_=sints_d[:, :])

                xt_r = xt_d.rearrange("(kc p) s -> p kc s", p=128)

                def rope(psum_q, n, dst, dst_cols):
                    """Apply rope to a [128, XW] projected chunk (2 heads) from
                    PSUM into dst[:, dst_cols] (SBUF)."""
                    c0, c1 = n * XW, (n + 1) * XW
                    for base in (0, 64):
                        tmph = rtmp_pool.tile([64, XW], F32, tag="swp", name="swp")
                        t2h = rtmp_pool.tile([64, XW], F32, tag="t2", name="t2")
                        t4h = rtmp_pool.tile([64, XW], F32, tag="t4", name="t4")
                        nc.vector.tensor_copy(
                            tmph[0:32, :], psum_q[base + 32:base + 64, :]
                        )
                        nc.vector.tensor_copy(
                            tmph[32:64, :], psum_q[base:base + 32, :]
                        )
                        nc.vector.tensor_mul(t2h, tmph, sints[:, c0:c1])
                        nc.vector.tensor_mul(
                            t4h, psum_q[base:base + 64, :], cost[:, c0:c1]
                        )
                        nc.vector.tensor_add(
                            dst[base:base + 64, dst_cols], t2h, t4h
                        )

                def attention(j):
                    nsk = 4 * (j + 1)
                    for m in range(NPAIR):
                        kt = kt_aa if m < 2 else kt_bb
                        va = vaug[m // 2]
                        qr = qrope.pop((m, j))
                        p2 = {}
                        for hb in (0, 64):  # head A at 0, head B at 64
                            p2[hb] = pout2_pool.tile([65, SQW], F32, tag="p2", name="p2")
                        pend = []  # staged (exp tile, hb, sk)
                        for sk in range(nsk):
                            ps = {}
                            for hb in (0, 64):
                                p1 = psc_pool.tile([128, SQW], F32, tag="sc", name="sc")
                                band = sk >= 4 * j
                                nc.tensor.matmul(
                                    p1,
                                    kt[hb:hb + 64, sk * 128:(sk + 1) * 128],
                                    qr[hb:hb + 64, :],
                                    start=True,
                                    stop=not band,
                                )
                                if band:
                                    r = sk - 4 * j
                                    if r > 0:
                                        nc.tensor.matmul(
                                            p1[:, 0:r * 128],
                                            ones_row,
                                            big_row[:, 0:r * 128],
                                            start=False,
                                            stop=False,
                                            skip_group_check=True,
                                        )
                                    nc.tensor.matmul(
                                        p1[:, r * 128:(r + 1) * 128],
                                        ident,
                                        tri,
                                        start=False,
                                        stop=True,
                                        skip_group_check=True,
                                    )
                                ps[hb] = p1
                            # exp then (staggered by one sk) the PV matmul
                            for hb in (0, 64):
                                et = expt_pool.tile([128, SQW], F32, tag="et", name="et")
                                nc.scalar.activation(et, ps[hb], AF.Exp)
                                pend.append((et, hb, sk))
                            while len(pend) > 2 or (sk == nsk - 1 and pend):
                                et, hb, psk = pend.pop(0)
                                nc.tensor.matmul(
                                    p2[hb],
                                    va[:, psk, :],
                                    et,
                                    start=(psk == 0),
                                    stop=(psk == nsk - 1),
                                    skip_group_check=True,
                                )
                        # normalize: attnT[hb..] = out^T * (1/sums) broadcast
                        at = attnt_pool.tile([128, SQW], F32, tag="at", name="at")
                        for hb in (0, 64):
                            inv = inv_pool.tile([1, SQW], F32, tag="inv", name="inv")
                            nc.vector.reciprocal(inv, p2[hb][64:65, :])
                            pb = pbc_pool.tile([64, SQW], F32, tag="bc", name="bc")
                            nc.tensor.matmul(
                                pb, ones_row[0:1, 0:64], inv, start=True, stop=True
                            )
                            pbs = inv_pool.tile([64, SQW], F32, tag="pbs", name="pbs")
                            nc.vector.tensor_copy(pbs, pb)
                            nc.vector.tensor_mul(
                                at[hb:hb + 64, :], p2[hb][0:64, :], pbs
                            )
                        attnt[(m, j)] = at

                for n in range(NN):
                    xt_t = {}
                    for kh in range(2):
                        xt_t[kh] = xt_pool.tile([128, NK // 2, XW], F32, tag="xt", name="xtt")
                        nc.sync.dma_start(
                            out=xt_t[kh],
                            in_=xt_r[:, kh * 8:(kh + 1) * 8, n * XW:(n + 1) * XW],
                        )
                    j, half = n // 2, n % 2
                    cols = slice(half * XW, (half + 1) * XW)
                    for m in range(6):  # 0-3: q pairs, 4: k, 5: v
                        pq = pqkv_pool.tile([128, XW], F32, tag="qkv", name="pqkv")
                        for k in range(NK):
                            nc.tensor.matmul(
                                pq,
                                wqt[:, k, m * 128:(m + 1) * 128],
                                xt_t[k // 8][:, k % 8, :],
                                start=(k == 0),
                                stop=(k == NK - 1),
                            )
                        if m < NPAIR:
                            if half == 0:
                                qrope[(m, j)] = qrope_pool.tile(
                                    [128, SQW], F32, tag="qr", name="qr"
                                )
                            rope(pq, n, qrope[(m, j)], cols)
                        elif m == 4:
                            kro = rtmp_pool.tile([128, XW], F32, tag="kro", name="kro")
                            rope(pq, n, kro, slice(0, XW))
                            c0, c1 = n * XW, (n + 1) * XW
                            for dst_b in (0, 64):
                                nc.vector.tensor_copy(
                                    kt_aa[dst_b:dst_b + 64, c0:c1], kro[0:64, :]
                                )
                                nc.vector.tensor_copy(
                                    kt_bb[dst_b:dst_b + 64, c0:c1], kro[64:128, :]
                                )
                        else:
                            vt = vt_pool.tile([128, XW], F32, tag="vt", name="vt")
                            nc.vector.tensor_copy(vt, pq)
                            for h2 in range(XW // 128):
                                sk = (n * XW) // 128 + h2
                                pt = pqkv_pool.tile([128, 128], F32, tag="qkv", name="pqkv")
                                nc.tensor.transpose(
                                    pt, vt[:, h2 * 128:(h2 + 1) * 128], ident
                                )
                                for i in range(NKV):
                                    nc.vector.tensor_copy(
                                        vaug[i][:, sk, 0:64], pt[:, i * 64:(i + 1) * 64]
                                    )
                    if half == 1:
                        attention(j)

            # ================= era 2: o_proj =================
            with (
                tc.tile_pool(name="wo", bufs=1) as wo_pool,
                tc.tile_pool(name="ost", bufs=4) as ost_pool,
                tc.tile_pool(name="pop", bufs=2, space="PSUM") as pop_pool,
            ):
                wot = wo_pool.tile([128, NPAIR, HID], F32, name="wot")
                for m in range(NPAIR):
                    nc.sync.dma_start(
                        out=wot[:, m, :], in_=wo_d[m * 128:(m + 1) * 128, :]
                    )
                for j in range(NJ):
                    for sc in range(SQW // 128):
                        for hc in range(HID // 512):
                            po = pop_pool.tile([128, 512], F32, tag="po", name="po")
                            for m in range(NPAIR):
                                nc.tensor.matmul(
                                    po,
                                    attnt[(m, j)][:, sc * 128:(sc + 1) * 128],
                                    wot[:, m, hc * 512:(hc + 1) * 512],
                                    start=(m == 0),
                                    stop=(m == NPAIR - 1),
                                )
                            ot = ost_pool.tile([128, 512], F32, tag="ot", name="ot")
                            nc.vector.tensor_copy(ot, po)
                            r0 = j * SQW + sc * 128
                            nc.sync.dma_start(
                                out=out_d[r0:r0 + 128, hc * 512:(hc + 1) * 512],
                                in_=ot,
                            )

    nc.finalize()
    _CACHED["nc"] = nc
    return nc


def _prep_inputs(cos, sin, hidden_states, per_head_scale, Wqkv, Wo):
    """Build the 8 per-core input maps (host-side, free)."""
    cos = np.asarray(cos, np.float32)
    sin = np.asarray(sin, np.float32)
    hs = np.asarray(hidden_states, np.float32)
    phs = np.asarray(per_head_scale, np.float32)
    Wqkv = np.asarray(Wqkv, np.float32)
    Wo = np.asarray(Wo, np.float32)

    cost = np.ascontiguousarray(cos.T)
    st = sin.T.copy()
    st[0:32] *= -1.0
    sints = np.ascontiguousarray(st)

    ident = np.eye(128, dtype=np.float32)
    tri = np.zeros((128, 128), np.float32)
    for p in range(128):
        tri[p, :p] = -BIG
    mrow = np.zeros((128, 640), np.float32)
    mrow[0, 0:128] = 1.0
    mrow[0, 128:640] = -BIG
    consts = np.ascontiguousarray(np.concatenate([ident, tri, mrow], axis=1))

    in_maps = []
    for c in range(8):
        b, g = c // 4, c % 4
        hq0 = NH * g
        wq = Wqkv[hq0 * D:(hq0 + NH) * D, :].copy()
        for h in range(NH):
            wq[h * D:(h + 1) * D] *= (
                ROPE_MSCALE * SM_SCALE * phs[b, hq0 + h]
            )
        kv0 = H * D + NKV * g * D
        wk = Wqkv[kv0:kv0 + NKV * D, :] * ROPE_MSCALE
        v0 = (H + K) * D + NKV * g * D
        wv = Wqkv[v0:v0 + NKV * D, :]
        wqkv_c = np.ascontiguousarray(np.concatenate([wq, wk, wv], axis=0).T)
        in_maps.append({
            "xt": np.ascontiguousarray(hs[b].T),
            "wqkv": wqkv_c,
            "wo": np.ascontiguousarray(Wo[:, hq0 * D:(hq0 + NH) * D].T),
            "cost": cost,
            "sints": sints,
            "consts": consts,
        })
    return in_maps


def kernel(cos, sin, hidden_states, per_head_scale, Wqkv, Wo, _trace=False):
    nc = _build()
    in_maps = _prep_inputs(cos, sin, hidden_states, per_head_scale, Wqkv, Wo)
    res = run_bass_kernel_spmd(nc, in_maps, core_ids=list(range(8)), trace=_trace)
    _CACHED["last_results"] = res
    out = np.stack([
        sum(res.results[b * 4 + g]["out"].astype(np.float64) for g in range(4))
        for b in range(B)
    ]).astype(np.float32)
    return out
